# revision 10
# baseline (speedup 1.0000x reference)
"""Trainium2 Bass kernel for nn_AttentionBlock (B=32, F=2048, H=W=7, A=1).

Math (reference):
  xf = x.reshape(B, F, 49)
  q, k, v = split(xf @ W_qkv.T)           # each [B, F, 49]
  S = (q @ k.T) / 7                       # [B, F, F]
  P = softmax(S, axis=-1)
  out = (P @ v) @ W_out.T + b_out         # [B, F, 1]
  out = batchnorm(out, axis=(0, 2)) * gamma + beta

Because A == 1 the output projection commutes into the attention sum:
  w[g]   = v[g] . W_out[0] = xf[g] . u,   u = W_v.T @ W_out[0]   (49-vector)
  out[f] = (sum_g E[f,g] * w[g]) / (sum_g E[f,g]) + b_out,  E = exp(S)
so the device only computes, per (batch, f): the weighted sum and the
denominator.  exp() is computed without max-subtraction (scores are O(1),
|s| < ~15, safely inside fp32 exp range), which keeps softmax a pure
free-axis-less dataflow:

  per batch:
    xfT [49, 2048]   via PE transposes of xf tiles
    qT = WqT.T @ xfT, kT = WkT.T @ xfT      [49, 2048]
    w  = xfT.T @ u                           [2048]
    per g-chunk (16 x 128 rows of the score matrix, transposed):
      ST[g128, f1024] = kT_slice.T @ qT      (2 matmuls into 2 PSUM banks)
      E = exp(ST / 7)                        (one ScalarE op per [128,1024])
      o[2, f512] += [w|1].T @ E              (PE accumulation, 4 f-chunks
                                              packed into one PSUM bank at
                                              partition offsets 0/32/64/96)

Batch is data-parallel across the 8 cores (4 batches each); the final
division, bias and the (exact, sync) BatchNorm run on host over the tiny
[32, 2048] result.

Matmul dtype is float32r (TF32-like, 1 col/cycle) by default; set
MM_DTYPE = "float32" for exact-fp32 (4x slower PE).
"""

import numpy as np
from contextlib import ExitStack

import concourse.bass as bass
import concourse.tile as tile
from concourse import bacc, mybir
from concourse import bass_utils

# Problem constants (hardcoded; harness provides full inputs).
B, F, HW, A = 32, 2048, 49, 1
N_CORES = 8
BPC = B // N_CORES          # batches per core
SCALE = 1.0 / 7.0           # hw ** -0.5
EPS = 1e-5

NF = F // 128               # 16 g-chunks of 128
NFC = F // 512              # 4 f-chunks of 512
HALF = 1024                 # exp tile width (2 PSUM banks)

MM_DT = mybir.dt.float32r   # matmul streaming dtype


def _build():
    nc = bacc.Bacc(
        "TRN2",
        target_bir_lowering=False,
        debug=False,
        num_devices=N_CORES,
    )
    f32 = mybir.dt.float32

    xs_d = nc.dram_tensor("xs", [BPC, F, HW], f32, kind="ExternalInput").ap()
    wqT_d = nc.dram_tensor("wqT", [HW, HW], MM_DT, kind="ExternalInput").ap()
    wkT_d = nc.dram_tensor("wkT", [HW, HW], MM_DT, kind="ExternalInput").ap()
    u_d = nc.dram_tensor("u", [HW, 2], MM_DT, kind="ExternalInput").ap()
    ones_d = nc.dram_tensor("ones", [128, 1], MM_DT, kind="ExternalInput").ap()
    id_d = nc.dram_tensor("ident", [128, 128], f32, kind="ExternalInput").ap()
    # res[b, 0, :] = weighted sums, res[b, 1, :] = denominators
    res_d = nc.dram_tensor("res", [BPC, 2, F], f32, kind="ExternalOutput").ap()

    with tile.TileContext(nc) as tc:
        with ExitStack() as ctx:
            wpool = ctx.enter_context(tc.tile_pool(name="wpool", bufs=1))
            xfp = ctx.enter_context(tc.tile_pool(name="xfp", bufs=2))
            xftp = ctx.enter_context(tc.tile_pool(name="xftp", bufs=2))
            qtp = ctx.enter_context(tc.tile_pool(name="qtp", bufs=2))
            ktp = ctx.enter_context(tc.tile_pool(name="ktp", bufs=2))
            vwp = ctx.enter_context(tc.tile_pool(name="vwp", bufs=2))
            ep = ctx.enter_context(tc.tile_pool(name="ep", bufs=3))
            resp = ctx.enter_context(tc.tile_pool(name="resp", bufs=2))
            pqp = ctx.enter_context(tc.tile_pool(name="pqp", bufs=2, space="PSUM"))
            stp = ctx.enter_context(tc.tile_pool(name="stp", bufs=2, space="PSUM"))
            op = ctx.enter_context(tc.tile_pool(name="op", bufs=1, space="PSUM"))

            wq_t = wpool.tile([HW, HW], MM_DT)
            wk_t = wpool.tile([HW, HW], MM_DT)
            u_t = wpool.tile([HW, 2], MM_DT)
            id_t = wpool.tile([128, 128], f32)
            nc.sync.dma_start(out=wq_t[:], in_=wqT_d)
            nc.sync.dma_start(out=wk_t[:], in_=wkT_d)
            nc.sync.dma_start(out=u_t[:], in_=u_d)
            nc.sync.dma_start(out=id_t[:], in_=id_d)

            for b in range(BPC):
                # ---- phase Q: load + transpose + project -------------------
                xf = xfp.tile([128, NF * HW], f32, tag="xf")
                nc.sync.dma_start(
                    out=xf[:].rearrange("p (t d) -> p t d", d=HW),
                    in_=xs_d[b].rearrange("(t p) d -> p t d", p=128),
                )
                xfT = xftp.tile([HW, F], MM_DT, tag="xfT")
                for j in range(NFC):  # one PSUM bank's worth: 4 transposes
                    tp = pqp.tile([HW, 512], f32, tag="pq")
                    for tt in range(4):
                        t = 4 * j + tt
                        nc.tensor.transpose(
                            tp[:, 512 * 0 + 128 * tt : 128 * tt + 128],
                            xf[:, t * HW : (t + 1) * HW],
                            id_t[:],
                        )
                    nc.vector.tensor_copy(xfT[:, j * 512 : (j + 1) * 512], tp[:])

                qT = qtp.tile([HW, F], MM_DT, tag="qT")
                kT = ktp.tile([HW, F], MM_DT, tag="kT")
                for j in range(NFC):
                    qp = pqp.tile([HW, 512], f32, tag="pq")
                    nc.tensor.matmul(
                        qp[:], wq_t[:], xfT[:, j * 512 : (j + 1) * 512],
                        start=True, stop=True,
                    )
                    nc.vector.tensor_copy(qT[:, j * 512 : (j + 1) * 512], qp[:])
                    kp = pqp.tile([HW, 512], f32, tag="pq")
                    nc.tensor.matmul(
                        kp[:], wk_t[:], xfT[:, j * 512 : (j + 1) * 512],
                        start=True, stop=True,
                    )
                    nc.vector.tensor_copy(kT[:, j * 512 : (j + 1) * 512], kp[:])

                # vw[:, g] = w_g for g < 16; vw[:, 16] = 1.0.  The O-matmul
                # stationary operand [w_g | 1] is the strided 2-element AP
                # vw[:, g:17:16-g].
                vw = vwp.tile([128, NF + 1], MM_DT, tag="vw")
                nc.sync.dma_start(out=vw[:, NF : NF + 1], in_=ones_d)
                for g in range(NF):
                    wp = pqp.tile([128, 2], f32, tag="pq")
                    nc.tensor.matmul(
                        wp[:], xfT[:, g * 128 : (g + 1) * 128], u_t[:],
                        start=True, stop=True,
                    )
                    nc.vector.tensor_copy(vw[:, g : g + 1], wp[:, 0:1])

                # ---- phase S: scores -> exp -> weighted sums ---------------
                res_sb = resp.tile([2, F], f32, tag="res")
                for h in range(F // HALF):
                    o_ps = [
                        op.tile([2, 512], f32, tag=f"o{q2}", name=f"o_ps{q2}_{b}_{h}")
                        for q2 in range(HALF // 512)
                    ]
                    for g in range(NF):
                        st = stp.tile([128, HALF], f32, tag="st")
                        for q2 in range(HALF // 512):
                            nc.tensor.matmul(
                                st[:, q2 * 512 : (q2 + 1) * 512],
                                kT[:, g * 128 : (g + 1) * 128],
                                qT[:, h * HALF + q2 * 512 : h * HALF + (q2 + 1) * 512],
                                start=True, stop=True,
                            )
                        e = ep.tile([128, HALF], MM_DT, tag="e")
                        nc.scalar.activation(
                            e[:], st[:], mybir.ActivationFunctionType.Exp,
                            scale=SCALE,
                        )
                        for q2 in range(HALF // 512):
                            nc.tensor.matmul(
                                o_ps[q2][:],
                                vw[:, g : NF + 1 : NF - g],
                                e[:, q2 * 512 : (q2 + 1) * 512],
                                start=(g == 0), stop=(g == NF - 1),
                            )
                    for q2 in range(HALF // 512):
                        fc = h * (HALF // 512) + q2
                        nc.vector.tensor_copy(
                            res_sb[:, fc * 512 : (fc + 1) * 512], o_ps[q2][:]
                        )
                nc.sync.dma_start(out=res_d[b], in_=res_sb[:])

    nc.compile()
    return nc


_NC = None


def _get_nc():
    global _NC
    if _NC is None:
        _NC = _build()
    return _NC


def _run(x, W_qkv, W_out, b_out, gamma, beta, trace=False):
    x = np.asarray(x, dtype=np.float32)
    W_qkv = np.asarray(W_qkv, dtype=np.float32)
    W_out = np.asarray(W_out, dtype=np.float32)
    b_out = np.asarray(b_out, dtype=np.float32)
    gamma = np.asarray(gamma, dtype=np.float32)
    beta = np.asarray(beta, dtype=np.float32)

    xf = np.ascontiguousarray(x.reshape(B, F, HW))
    wqT = np.ascontiguousarray(W_qkv[0:HW].T)
    wkT = np.ascontiguousarray(W_qkv[HW : 2 * HW].T)
    u = np.zeros((HW, 2), dtype=np.float32)
    u[:, 0] = W_qkv[2 * HW : 3 * HW].T @ W_out[0]
    ones = np.ones((128, 1), dtype=np.float32)
    ident = np.eye(128, dtype=np.float32)

    in_maps = []
    for c in range(N_CORES):
        in_maps.append(
            {
                "xs": np.ascontiguousarray(xf[c * BPC : (c + 1) * BPC]),
                "wqT": wqT,
                "wkT": wkT,
                "u": u,
                "ones": ones,
                "ident": ident,
            }
        )

    nc = _get_nc()
    res = bass_utils.run_bass_kernel_spmd(
        nc, in_maps, core_ids=list(range(N_CORES)), trace=trace
    )

    # Assemble [B, F]: weighted sum / denominator + bias.
    outs = np.empty((B, F), dtype=np.float64)
    for c in range(N_CORES):
        r = res.results[c]["res"]  # [BPC, 2, F]
        num = r[:, 0, :].astype(np.float64)
        den = r[:, 1, :].astype(np.float64)
        outs[c * BPC : (c + 1) * BPC] = num / den
    outs += np.float64(b_out[0])

    # Sync BatchNorm over the batch axis (A == 1 so stats are per-channel
    # over the 32 batch values), exactly as the reference computes it.
    mean = outs.mean(axis=0, keepdims=True)
    var = ((outs - mean) ** 2).mean(axis=0, keepdims=True)
    outn = (outs - mean) / np.sqrt(var + EPS)
    outn = outn * gamma[None, :].astype(np.float64) + beta[None, :].astype(np.float64)
    return outn.astype(np.float32).reshape(B, F, A), res


def kernel(x, W_qkv, W_out, b_out, gamma, beta):
    out, _ = _run(x, W_qkv, W_out, b_out, gamma, beta, trace=False)
    return out


# revision 13
# speedup vs baseline: 1.9661x; 1.9661x over previous
"""Trainium2 Bass kernel for nn_AttentionBlock (B=32, F=2048, H=W=7, A=1).

Math (reference):
  xf = x.reshape(B, F, 49)
  q, k, v = split(xf @ W_qkv.T)           # each [B, F, 49]
  S = (q @ k.T) / 7                       # [B, F, F]
  P = softmax(S, axis=-1)
  out = (P @ v) @ W_out.T + b_out         # [B, F, 1]
  out = batchnorm(out, axis=(0, 2)) * gamma + beta

Because A == 1 the output projection commutes into the attention sum:
  w[g]   = v[g] . W_out[0] = xf[g] . u,   u = W_v.T @ W_out[0]   (49-vector)
  out[f] = (sum_g E[f,g] * w[g]) / (sum_g E[f,g]) + b_out,  E = exp(S)
so the device only computes, per (batch, f), the weighted sum and the
denominator.  exp() is computed without max-subtraction (scores are O(1),
|s| < ~15, safely inside fp32 exp range).

Layout: scores are built TRANSPOSED (S_T[g,f] tiles, g on partitions) so
both reductions over g are PE matmuls with a tiny [w_g | 1] stationary:

  per batch:
    xfT [49, 2048]  via PE transposes; duplicated at partitions 64..112
    qT/kT = W{q,k}T.T @ xfT  [49, 2048], duplicated at partitions 64..112
    vw[:, g] = w_g (g<16), vw[:, 16] = 1
    per f-half h (1024 wide), per g-chunk (16 x 128):
      ST[g128, f1024] = kT_g.T @ qT   (2 MMs in row groups 0 / 64 -> run
                                       concurrently in the PE array)
      E = exp(ST / 7)                 (one ScalarE op, reads 2 PSUM banks)
      o[q2] += [w_g | 1].T @ E_half   (accumulating MMs, M=2)

Phase-Q work for batch b+1 (DMA, transposes, projections) is emitted
interleaved into batch b's score loop so the PE never idles waiting on
exp(): idle gaps cause HAM re-throttling to 1.2 GHz (measured 2x).

Batch is data-parallel across the 8 cores (4 batches each); the final
division, bias and the (exact, sync) BatchNorm run on host over the tiny
[32, 2048] result.  Matmuls use float32r (TF32-like single pass).
"""

import numpy as np
from contextlib import ExitStack

import concourse.bass as bass
import concourse.tile as tile
from concourse import bacc, mybir, bass_utils

B, F, HW, A = 32, 2048, 49, 1
N_CORES = 8
BPC = B // N_CORES
SCALE = 1.0 / 7.0
EPS = 1e-5

NF = F // 128               # 16 g-chunks
HALF = 1024
NH = F // HALF              # 2 f-halves
MM_DT = mybir.dt.float32r


def _build():
    nc = bacc.Bacc(
        "TRN2",
        target_bir_lowering=False,
        debug=False,
        num_devices=N_CORES,
    )
    f32 = mybir.dt.float32

    xs_d = nc.dram_tensor("xs", [BPC, F, HW], f32, kind="ExternalInput").ap()
    wqT_d = nc.dram_tensor("wqT", [HW, HW], MM_DT, kind="ExternalInput").ap()
    wkT_d = nc.dram_tensor("wkT", [HW, HW], MM_DT, kind="ExternalInput").ap()
    u_d = nc.dram_tensor("u", [HW, 2], MM_DT, kind="ExternalInput").ap()
    ones_d = nc.dram_tensor("ones", [128, 1], MM_DT, kind="ExternalInput").ap()
    id_d = nc.dram_tensor("ident", [128, 128], f32, kind="ExternalInput").ap()
    res_d = nc.dram_tensor("res", [BPC, 2, F], f32, kind="ExternalOutput").ap()

    with tile.TileContext(nc) as tc:
        with ExitStack() as ctx:
            wpool = ctx.enter_context(tc.tile_pool(name="wpool", bufs=1))
            xfp = ctx.enter_context(tc.tile_pool(name="xfp", bufs=2))
            xftp = ctx.enter_context(tc.tile_pool(name="xftp", bufs=2))
            qtp = ctx.enter_context(tc.tile_pool(name="qtp", bufs=2))
            ktp = ctx.enter_context(tc.tile_pool(name="ktp", bufs=2))
            vwp = ctx.enter_context(tc.tile_pool(name="vwp", bufs=2))
            ep = ctx.enter_context(tc.tile_pool(name="ep", bufs=4))
            resp = ctx.enter_context(tc.tile_pool(name="resp", bufs=2))
            pqp = ctx.enter_context(tc.tile_pool(name="pqp", bufs=2, space="PSUM"))
            stp = ctx.enter_context(tc.tile_pool(name="stp", bufs=2, space="PSUM"))
            op = ctx.enter_context(tc.tile_pool(name="op", bufs=1, space="PSUM"))

            wq_t = wpool.tile([HW, HW], MM_DT)
            wk_t = wpool.tile([HW, HW], MM_DT)
            u_t = wpool.tile([HW, 2], MM_DT)
            id_t = wpool.tile([128, 128], f32)
            nc.sync.dma_start(out=wq_t[:], in_=wqT_d)
            nc.sync.dma_start(out=wk_t[:], in_=wkT_d)
            nc.sync.dma_start(out=u_t[:], in_=u_d)
            nc.sync.dma_start(out=id_t[:], in_=id_d)

            state = {}

            def phase_q_steps(b):
                """Generator yielding phase-Q work for batch b in small slices.

                Each yielded call emits a few PE ops (plus their DVE copies)
                so the caller can interleave them into another batch's score
                loop, keeping the PE stream dense.
                """
                xf = xfp.tile([128, NF * HW], f32, tag="xf", name=f"xf{b}")
                nc.sync.dma_start(
                    out=xf[:].rearrange("p (t d) -> p t d", d=HW),
                    in_=xs_d[b].rearrange("(t p) d -> p t d", p=128),
                )
                xfT = xftp.tile([HW, F], MM_DT, tag="xfT", name=f"xfT{b}")
                qT = qtp.tile([128, F], MM_DT, tag="qT", name=f"qT{b}")
                kT = ktp.tile([128, F], MM_DT, tag="kT", name=f"kT{b}")
                vw = vwp.tile([128, NF + 1], MM_DT, tag="vw", name=f"vw{b}")
                nc.sync.dma_start(out=vw[:, NF : NF + 1], in_=ones_d)
                state[b] = (qT, kT, vw)
                yield
                # transposes: 4 slices of (4 transposes + 1 copy)
                for j in range(4):
                    tp = pqp.tile([HW, 512], f32, tag="pq", name=f"tp{b}_{j}")
                    for tt in range(4):
                        t = 4 * j + tt
                        nc.tensor.transpose(
                            tp[:, 128 * tt : 128 * tt + 128],
                            xf[:, t * HW : (t + 1) * HW],
                            id_t[:],
                        )
                    nc.vector.tensor_copy(xfT[:, j * 512 : (j + 1) * 512], tp[:])
                    yield
                # q/k projections: 8 slices of (1 MM + 2 copies)
                for j in range(4):
                    sl = slice(j * 512, (j + 1) * 512)
                    qp = pqp.tile([HW, 512], f32, tag="pq", name=f"qp{b}_{j}")
                    nc.tensor.matmul(qp[:], wq_t[:], xfT[:, sl], start=True, stop=True)
                    nc.vector.tensor_copy(qT[0:HW, sl], qp[:])
                    nc.vector.tensor_copy(qT[64 : 64 + HW, sl], qp[:])
                    yield
                    kp = pqp.tile([HW, 512], f32, tag="pq", name=f"kp{b}_{j}")
                    nc.tensor.matmul(kp[:], wk_t[:], xfT[:, sl], start=True, stop=True)
                    nc.vector.tensor_copy(kT[0:HW, sl], kp[:])
                    nc.vector.tensor_copy(kT[64 : 64 + HW, sl], kp[:])
                    yield
                # w: 16 slices of (1 MM + 1 copy)
                for g in range(NF):
                    wp = pqp.tile([128, 2], f32, tag="pq", name=f"wp{b}_{g}")
                    nc.tensor.matmul(
                        wp[:], xfT[:, g * 128 : (g + 1) * 128], u_t[:],
                        start=True, stop=True,
                    )
                    nc.vector.tensor_copy(vw[:, g : g + 1], wp[:, 0:1])
                    yield
                while True:
                    yield

            def phase_s(b, qnext):
                """Score loop for batch b, pulling phase-Q slices of b+1."""
                qT, kT, vw = state[b]
                res_sb = resp.tile([2, F], f32, tag="res", name=f"res{b}")
                for h in range(NH):
                    o_ps = [
                        op.tile([2, 512], f32, tag=f"o{q2}", name=f"o{q2}_{b}_{h}")
                        for q2 in range(2)
                    ]
                    es = {}
                    for g in range(NF + 1):
                        if g < NF:
                            st = stp.tile([128, HALF], f32, tag="st", name=f"st{b}_{h}_{g}")
                            for q2 in range(2):
                                base = 64 * q2
                                nc.tensor.matmul(
                                    st[:, q2 * 512 : (q2 + 1) * 512],
                                    kT[base : base + HW, g * 128 : (g + 1) * 128],
                                    qT[base : base + HW,
                                       h * HALF + q2 * 512 : h * HALF + (q2 + 1) * 512],
                                    start=True, stop=True, tile_position=(base, 0),
                                )
                        if g >= 1:
                            gp = g - 1
                            for q2 in range(2):
                                nc.tensor.matmul(
                                    o_ps[q2][:],
                                    vw[:, gp : NF + 1 : NF - gp],
                                    es[gp][:, q2 * 512 : (q2 + 1) * 512],
                                    start=(gp == 0), stop=(gp == NF - 1),
                                )
                        if g < NF:
                            e = ep.tile([128, HALF], MM_DT, tag="e", name=f"e{b}_{h}_{g}")
                            es[g] = e
                            nc.scalar.activation(
                                e[:], st[:], mybir.ActivationFunctionType.Exp,
                                scale=SCALE,
                            )
                        if qnext is not None:
                            next(qnext)
                    for q2 in range(2):
                        fc = h * 2 + q2
                        nc.vector.tensor_copy(
                            res_sb[:, fc * 512 : (fc + 1) * 512], o_ps[q2][:]
                        )
                nc.sync.dma_start(out=res_d[b], in_=res_sb[:])

            q0 = phase_q_steps(0)
            for _ in range(30):
                next(q0)
            for b in range(BPC):
                qnext = phase_q_steps(b + 1) if b + 1 < BPC else None
                if qnext is not None:
                    next(qnext)  # DMA + tile allocation upfront
                phase_s(b, qnext)

    nc.compile()
    return nc


_NC = None


def _get_nc():
    global _NC
    if _NC is None:
        _NC = _build()
    return _NC


def _run(x, W_qkv, W_out, b_out, gamma, beta, trace=False):
    x = np.asarray(x, dtype=np.float32)
    W_qkv = np.asarray(W_qkv, dtype=np.float32)
    W_out = np.asarray(W_out, dtype=np.float32)
    b_out = np.asarray(b_out, dtype=np.float32)
    gamma = np.asarray(gamma, dtype=np.float32)
    beta = np.asarray(beta, dtype=np.float32)

    xf = np.ascontiguousarray(x.reshape(B, F, HW))
    wqT = np.ascontiguousarray(W_qkv[0:HW].T)
    wkT = np.ascontiguousarray(W_qkv[HW : 2 * HW].T)
    u = np.zeros((HW, 2), dtype=np.float32)
    u[:, 0] = W_qkv[2 * HW : 3 * HW].T @ W_out[0]
    ones = np.ones((128, 1), dtype=np.float32)
    ident = np.eye(128, dtype=np.float32)

    in_maps = []
    for c in range(N_CORES):
        in_maps.append(
            {
                "xs": np.ascontiguousarray(xf[c * BPC : (c + 1) * BPC]),
                "wqT": wqT,
                "wkT": wkT,
                "u": u,
                "ones": ones,
                "ident": ident,
            }
        )

    nc = _get_nc()
    res = bass_utils.run_bass_kernel_spmd(
        nc, in_maps, core_ids=list(range(N_CORES)), trace=trace
    )

    outs = np.empty((B, F), dtype=np.float64)
    for c in range(N_CORES):
        r = res.results[c]["res"]  # [BPC, 2, F]
        num = r[:, 0, :].astype(np.float64)
        den = r[:, 1, :].astype(np.float64)
        outs[c * BPC : (c + 1) * BPC] = num / den
    outs += np.float64(b_out[0])

    # Sync BatchNorm over the batch axis, exactly as the reference.
    mean = outs.mean(axis=0, keepdims=True)
    var = ((outs - mean) ** 2).mean(axis=0, keepdims=True)
    outn = (outs - mean) / np.sqrt(var + EPS)
    outn = outn * gamma[None, :].astype(np.float64) + beta[None, :].astype(np.float64)
    return outn.astype(np.float32).reshape(B, F, A), res


def kernel(x, W_qkv, W_out, b_out, gamma, beta):
    out, _ = _run(x, W_qkv, W_out, b_out, gamma, beta, trace=False)
    return out


# revision 19
# speedup vs baseline: 1.9940x; 1.0142x over previous
"""Trainium2 Bass kernel for nn_AttentionBlock (B=32, F=2048, H=W=7, A=1).

Math (reference):
  xf = x.reshape(B, F, 49)
  q, k, v = split(xf @ W_qkv.T)           # each [B, F, 49]
  S = (q @ k.T) / 7                       # [B, F, F]
  P = softmax(S, axis=-1)
  out = (P @ v) @ W_out.T + b_out         # [B, F, 1]
  out = batchnorm(out, axis=(0, 2)) * gamma + beta

Because A == 1 the output projection commutes into the attention sum:
  w[g]   = v[g] . W_out[0] = xf[g] . u,   u = W_v.T @ W_out[0]   (49-vector)
  out[f] = (sum_g E[f,g] * w[g]) / (sum_g E[f,g]) + b_out,  E = exp(S)
so the device only computes, per (batch, f), the weighted sum and the
denominator.  exp() is computed without max-subtraction (scores are O(1),
|s| < ~15, safely inside fp32 exp range).

Layout: scores are built TRANSPOSED (S_T[g,f] tiles, g on partitions) so
both reductions over g are PE matmuls with a tiny [w_g | 1] stationary:

  per batch:
    xfT [49, 2048]  via PE transposes; duplicated at partitions 64..112
    qT/kT = W{q,k}T.T @ xfT  [49, 2048], duplicated at partitions 64..112
    vw[:, g] = w_g (g<16), vw[:, 16] = 1
    per f-half h (1024 wide), per g-chunk (16 x 128):
      ST[g128, f1024] = kT_g.T @ qT   (2 MMs in row groups 0 / 64 -> run
                                       concurrently in the PE array)
      E = exp(ST / 7)                 (one ScalarE op, reads 2 PSUM banks)
      o[q2] += [w_g | 1].T @ E_half   (accumulating MMs, M=2)

Phase-Q work for batch b+1 (DMA, transposes, projections) is emitted
interleaved into batch b's score loop so the PE never idles waiting on
exp(): idle gaps cause HAM re-throttling to 1.2 GHz (measured 2x).

Batch is data-parallel across the 8 cores (4 batches each); the final
division, bias and the (exact, sync) BatchNorm run on host over the tiny
[32, 2048] result.  Matmuls use float32r (TF32-like single pass).
"""

import numpy as np
from contextlib import ExitStack

import concourse.bass as bass
import concourse.tile as tile
from concourse import bacc, mybir, bass_utils

B, F, HW, A = 32, 2048, 49, 1
N_CORES = 8
BPC = B // N_CORES
SCALE = 1.0 / 7.0
EPS = 1e-5

NF = F // 128               # 16 g-chunks
HALF = 1024
NH = F // HALF              # 2 f-halves
MM_DT = mybir.dt.float32r


def _build():
    nc = bacc.Bacc(
        "TRN2",
        target_bir_lowering=False,
        debug=False,
        num_devices=N_CORES,
    )
    f32 = mybir.dt.float32

    xs_d = nc.dram_tensor("xs", [BPC, F, HW], f32, kind="ExternalInput").ap()
    wqT_d = nc.dram_tensor("wqT", [HW, HW], MM_DT, kind="ExternalInput").ap()
    wkT_d = nc.dram_tensor("wkT", [HW, HW], MM_DT, kind="ExternalInput").ap()
    u_d = nc.dram_tensor("u", [HW, 2], MM_DT, kind="ExternalInput").ap()
    ones_d = nc.dram_tensor("ones", [128, 1], MM_DT, kind="ExternalInput").ap()
    id_d = nc.dram_tensor("ident", [128, 128], f32, kind="ExternalInput").ap()
    res_d = nc.dram_tensor("res", [BPC, 2, F], f32, kind="ExternalOutput").ap()

    with tile.TileContext(nc) as tc:
        with ExitStack() as ctx:
            wpool = ctx.enter_context(tc.tile_pool(name="wpool", bufs=1))
            xfp = ctx.enter_context(tc.tile_pool(name="xfp", bufs=2))
            xftp = ctx.enter_context(tc.tile_pool(name="xftp", bufs=2))
            qtp = ctx.enter_context(tc.tile_pool(name="qtp", bufs=2))
            ktp = ctx.enter_context(tc.tile_pool(name="ktp", bufs=2))
            vwp = ctx.enter_context(tc.tile_pool(name="vwp", bufs=2))
            ep = ctx.enter_context(tc.tile_pool(name="ep", bufs=4))
            resp = ctx.enter_context(tc.tile_pool(name="resp", bufs=2))
            pqp = ctx.enter_context(tc.tile_pool(name="pqp", bufs=2, space="PSUM"))
            stp = ctx.enter_context(tc.tile_pool(name="stp", bufs=2, space="PSUM"))
            op = ctx.enter_context(tc.tile_pool(name="op", bufs=1, space="PSUM"))

            wq_t = wpool.tile([HW, HW], MM_DT)
            wk_t = wpool.tile([HW, HW], MM_DT)
            u_t = wpool.tile([HW, 2], MM_DT)
            id_t = wpool.tile([128, 128], f32)
            nc.sync.dma_start(out=wq_t[:], in_=wqT_d)
            nc.sync.dma_start(out=wk_t[:], in_=wkT_d)
            nc.sync.dma_start(out=u_t[:], in_=u_d)
            nc.sync.dma_start(out=id_t[:], in_=id_d)

            state = {}

            def phase_q_steps(b):
                """Generator yielding phase-Q work for batch b in small slices.

                Each yielded call emits a few PE ops (plus their DVE copies)
                so the caller can interleave them into another batch's score
                loop, keeping the PE stream dense.
                """
                xf = xfp.tile([128, NF * HW], f32, tag="xf", name=f"xf{b}")
                nc.sync.dma_start(
                    out=xf[:].rearrange("p (t d) -> p t d", d=HW),
                    in_=xs_d[b].rearrange("(t p) d -> p t d", p=128),
                )
                xfT = xftp.tile([HW, F], MM_DT, tag="xfT", name=f"xfT{b}")
                qT = qtp.tile([128, F], MM_DT, tag="qT", name=f"qT{b}")
                kT = ktp.tile([128, F], MM_DT, tag="kT", name=f"kT{b}")
                vw = vwp.tile([128, NF + 1], MM_DT, tag="vw", name=f"vw{b}")
                nc.sync.dma_start(out=vw[:, NF : NF + 1], in_=ones_d)
                state[b] = (qT, kT, vw)
                yield
                # transposes: 4 slices of (4 transposes + 1 copy)
                for j in range(4):
                    tp = pqp.tile([HW, 512], f32, tag="pq", name=f"tp{b}_{j}")
                    for tt in range(4):
                        t = 4 * j + tt
                        nc.tensor.transpose(
                            tp[:, 128 * tt : 128 * tt + 128],
                            xf[:, t * HW : (t + 1) * HW],
                            id_t[:],
                        )
                    nc.vector.tensor_copy(xfT[:, j * 512 : (j + 1) * 512], tp[:])
                    yield
                # q/k projections: 8 slices of (1 MM + 2 copies)
                for j in range(4):
                    sl = slice(j * 512, (j + 1) * 512)
                    qp = pqp.tile([HW, 512], f32, tag="pq", name=f"qp{b}_{j}")
                    nc.tensor.matmul(qp[:], wq_t[:], xfT[:, sl], start=True, stop=True)
                    nc.vector.tensor_copy(qT[0:HW, sl], qp[:])
                    nc.vector.tensor_copy(qT[64 : 64 + HW, sl], qp[:])
                    yield
                    kp = pqp.tile([HW, 512], f32, tag="pq", name=f"kp{b}_{j}")
                    nc.tensor.matmul(kp[:], wk_t[:], xfT[:, sl], start=True, stop=True)
                    nc.vector.tensor_copy(kT[0:HW, sl], kp[:])
                    nc.vector.tensor_copy(kT[64 : 64 + HW, sl], kp[:])
                    yield
                # w row -> column layout without DRAM: park the 4 row-chunks
                # at partitions {0,32,64,96} of w4, then PE-transpose each
                # 128-col block; chunk j lands in transpose-output column 32j.
                w4 = vwp.tile([97, 512], f32, tag="w4", name=f"w4_{b}")
                for j in range(4):
                    sl = slice(j * 512, (j + 1) * 512)
                    wp = pqp.tile([1, 512], f32, tag="pq", name=f"wp{b}_{j}")
                    nc.tensor.matmul(wp[:], u_t[:, 0:1], xfT[:, sl], start=True, stop=True)
                    nc.vector.tensor_copy(w4[32 * j : 32 * j + 1, :], wp[:])
                    yield
                for c in range(4):
                    tp = pqp.tile([128, 97], f32, tag="pq", name=f"wt{b}_{c}")
                    nc.tensor.transpose(
                        tp[:], w4[:, c * 128 : (c + 1) * 128], id_t[0:97, 0:97]
                    )
                    # column 32j of tp holds w for g-chunk 4j + c
                    nc.vector.tensor_copy(vw[:, c : NF : 4], tp[:, 0:97:32])
                    yield
                while True:
                    yield

            def phase_s(b, qnext):
                """Score loop for batch b, pulling phase-Q slices of b+1."""
                qT, kT, vw = state[b]
                res_sb = resp.tile([2, F], f32, tag="res", name=f"res{b}")
                for h in range(NH):
                    o_ps = [
                        op.tile([2, 512], f32, tag=f"o{q2}", name=f"o{q2}_{b}_{h}")
                        for q2 in range(2)
                    ]
                    es = {}
                    for g in range(NF + 1):
                        if g < NF:
                            st = stp.tile([128, HALF], f32, tag="st", name=f"st{b}_{h}_{g}")
                            for q2 in range(2):
                                base = 64 * q2
                                nc.tensor.matmul(
                                    st[:, q2 * 512 : (q2 + 1) * 512],
                                    kT[base : base + HW, g * 128 : (g + 1) * 128],
                                    qT[base : base + HW,
                                       h * HALF + q2 * 512 : h * HALF + (q2 + 1) * 512],
                                    start=True, stop=True, tile_position=(base, 0),
                                )
                        if g >= 1:
                            gp = g - 1
                            for q2 in range(2):
                                nc.tensor.matmul(
                                    o_ps[q2][:],
                                    vw[:, gp : NF + 1 : NF - gp],
                                    es[gp][:, q2 * 512 : (q2 + 1) * 512],
                                    start=(gp == 0), stop=(gp == NF - 1),
                                )
                        if g < NF:
                            e = ep.tile([128, HALF], MM_DT, tag="e", name=f"e{b}_{h}_{g}")
                            es[g] = e
                            nc.scalar.activation(
                                e[:], st[:], mybir.ActivationFunctionType.Exp,
                                scale=SCALE,
                            )
                        if qnext is not None:
                            next(qnext)
                    for q2 in range(2):
                        fc = h * 2 + q2
                        nc.vector.tensor_copy(
                            res_sb[:, fc * 512 : (fc + 1) * 512], o_ps[q2][:]
                        )
                nc.sync.dma_start(out=res_d[b], in_=res_sb[:])

            q0 = phase_q_steps(0)
            for _ in range(30):
                next(q0)
            for b in range(BPC):
                qnext = phase_q_steps(b + 1) if b + 1 < BPC else None
                if qnext is not None:
                    next(qnext)  # DMA + tile allocation upfront
                phase_s(b, qnext)

    nc.compile()
    return nc


_NC = None


def _get_nc():
    global _NC
    if _NC is None:
        _NC = _build()
    return _NC


def _run(x, W_qkv, W_out, b_out, gamma, beta, trace=False):
    x = np.asarray(x, dtype=np.float32)
    W_qkv = np.asarray(W_qkv, dtype=np.float32)
    W_out = np.asarray(W_out, dtype=np.float32)
    b_out = np.asarray(b_out, dtype=np.float32)
    gamma = np.asarray(gamma, dtype=np.float32)
    beta = np.asarray(beta, dtype=np.float32)

    xf = np.ascontiguousarray(x.reshape(B, F, HW))
    wqT = np.ascontiguousarray(W_qkv[0:HW].T)
    wkT = np.ascontiguousarray(W_qkv[HW : 2 * HW].T)
    u = np.zeros((HW, 2), dtype=np.float32)
    u[:, 0] = W_qkv[2 * HW : 3 * HW].T @ W_out[0]
    ones = np.ones((128, 1), dtype=np.float32)
    ident = np.eye(128, dtype=np.float32)

    in_maps = []
    for c in range(N_CORES):
        in_maps.append(
            {
                "xs": np.ascontiguousarray(xf[c * BPC : (c + 1) * BPC]),
                "wqT": wqT,
                "wkT": wkT,
                "u": u,
                "ones": ones,
                "ident": ident,
            }
        )

    nc = _get_nc()
    res = bass_utils.run_bass_kernel_spmd(
        nc, in_maps, core_ids=list(range(N_CORES)), trace=trace
    )

    outs = np.empty((B, F), dtype=np.float64)
    for c in range(N_CORES):
        r = res.results[c]["res"]  # [BPC, 2, F]
        num = r[:, 0, :].astype(np.float64)
        den = r[:, 1, :].astype(np.float64)
        outs[c * BPC : (c + 1) * BPC] = num / den
    outs += np.float64(b_out[0])

    # Sync BatchNorm over the batch axis, exactly as the reference.
    mean = outs.mean(axis=0, keepdims=True)
    var = ((outs - mean) ** 2).mean(axis=0, keepdims=True)
    outn = (outs - mean) / np.sqrt(var + EPS)
    outn = outn * gamma[None, :].astype(np.float64) + beta[None, :].astype(np.float64)
    return outn.astype(np.float32).reshape(B, F, A), res


def kernel(x, W_qkv, W_out, b_out, gamma, beta):
    out, _ = _run(x, W_qkv, W_out, b_out, gamma, beta, trace=False)
    return out
